# revision 1
# baseline (speedup 1.0000x reference)
"""Trainium2 Bass kernel for nn_CounterFlowNetwork.

Data-parallel over 8 NeuronCores (batch sharded), with the whole
counterflow sweep restructured to minimize matmul and elementwise work:

 - Consecutive linear layers folded host-side:
     delta @ W_ab -> liquid tracked in "equilibrium-projected" space
     (W_trabeq = alpha*W_tr @ W_ab @ W_eq), so a descending plate is ONE
     256x256 matmul instead of three.
 - The descending-sweep sigmoid at plate n-1 and the ascending-sweep
   sigmoid at plate n use the *same* l[n], so each sweep needs only 8
   sigmoid field evals instead of 16.
 - l[1] for the output head is recovered from S = sum of descending
   driving forces:  l1 @ W1_l = S @ (W_trab @ W1_l) + const.
 - All activations live transposed in SBUF ([feature, row]); the final
   head matmul uses h as the stationary operand so the output lands in
   natural [row, feature] layout for a clean DMA out.
 - Matmuls run in float32r (tf32-like, full PE rate); biases are folded
   into ACT activation biases or injected with K=1 ones-matmuls.
"""

import numpy as np

import concourse.bass as bass
import concourse.bacc as bacc
import concourse.mybir as mybir
import concourse.tile as tile
from concourse import bass_utils

B, D_IN, D_GAS, D_OUT = 16384, 512, 256, 1000
N_PLATES = 8
N_CORES = 8
ROWS = B // N_CORES          # rows per core
N_CHUNKS = 4
R = ROWS // N_CHUNKS         # rows per chunk
F32 = mybir.dt.float32
F32R = mybir.dt.float32r
AF = mybir.ActivationFunctionType
OP = mybir.AluOpType

# How many descending-df evals (out of 16 per chunk) go to GPSIMD
DESC_DF_ON_GPSIMD = {(0, 4), (0, 5), (1, 4)}  # (sweep, plate) pairs


def _preprocess_weights(inp):
    """Fold the plate linear algebra host-side (float64, cast to f32)."""
    f32, f64 = np.float32, np.float64
    W_tr = np.asarray(inp["W_tr"], f32)
    b_tr = np.asarray(inp["b_tr"], f32)
    W_ab = np.asarray(inp["W_ab"], f32)
    b_ab = np.asarray(inp["b_ab"], f32)
    W_eq = np.asarray(inp["W_eq"], f32)
    b_eq = np.asarray(inp["b_eq"], f32)
    W1 = np.asarray(inp["W1"], f32)
    b1 = np.asarray(inp["b1"], f32)
    alpha = float(np.asarray(inp["alpha"]))

    W_trp = (alpha * W_tr).astype(f32)
    ab_tr = (alpha * b_tr).astype(f32)
    W_trab = (W_trp.astype(f64) @ W_ab).astype(f32)
    c2 = (ab_tr.astype(f64) @ W_ab + b_ab).astype(f32)
    W_trabeq = (W_trab.astype(f64) @ W_eq).astype(f32)
    c3 = (c2.astype(f64) @ W_eq).astype(f32)
    W1_g, W1_l = W1[:D_GAS], W1[D_GAS:]
    W_fold = (W_trab.astype(f64) @ W1_l).astype(f32)
    bias_e = np.stack([(b_eq + (9 - n) * c3) for n in range(1, 9)]).astype(f32)
    e9 = (1.0 / (1.0 + np.exp(-b_eq.astype(f64)))).astype(f32)
    h_bias = (b1 + 8.0 * (c2.astype(f64) @ W1_l)).astype(f32)

    return {
        "wge": np.ascontiguousarray(np.asarray(inp["W_ge"], f32)),
        "wdesc": np.ascontiguousarray(W_trabeq),
        "wasc": np.ascontiguousarray(-W_trp),
        "wfold": np.ascontiguousarray(W_fold),
        "w1g": np.ascontiguousarray(W1_g),
        "w2": np.ascontiguousarray(np.asarray(inp["W2"], f32)),
        "nab": np.ascontiguousarray(-ab_tr.reshape(1, D_GAS)),
        "b2r": np.ascontiguousarray(np.asarray(inp["b2"], f32).reshape(1, D_OUT)),
        "be": np.ascontiguousarray(bias_e),          # (8, 256), index n-1
        "ne9": np.ascontiguousarray(-e9),            # (256,)
        "bge": np.ascontiguousarray(np.asarray(inp["b_ge"], f32)),
        "hb": np.ascontiguousarray(h_bias),
        "iden": np.eye(128, dtype=f32),
        "ones": np.ones((1, R), f32),
    }


def build_nc():
    nc = bacc.Bacc("TRN2", target_bir_lowering=False, debug=False)

    x_d = nc.dram_tensor("x", (ROWS, D_IN), F32R, kind="ExternalInput").ap()
    wge_d = nc.dram_tensor("wge", (D_IN, D_GAS), F32R, kind="ExternalInput").ap()
    wdesc_d = nc.dram_tensor("wdesc", (D_GAS, D_GAS), F32R, kind="ExternalInput").ap()
    wasc_d = nc.dram_tensor("wasc", (D_GAS, D_GAS), F32R, kind="ExternalInput").ap()
    wfold_d = nc.dram_tensor("wfold", (D_GAS, D_GAS), F32R, kind="ExternalInput").ap()
    w1g_d = nc.dram_tensor("w1g", (D_GAS, D_GAS), F32R, kind="ExternalInput").ap()
    w2_d = nc.dram_tensor("w2", (D_GAS, D_OUT), F32R, kind="ExternalInput").ap()
    nab_d = nc.dram_tensor("nab", (1, D_GAS), F32R, kind="ExternalInput").ap()
    b2r_d = nc.dram_tensor("b2r", (1, D_OUT), F32R, kind="ExternalInput").ap()
    be_d = nc.dram_tensor("be", (8, D_GAS), F32, kind="ExternalInput").ap()
    ne9_d = nc.dram_tensor("ne9", (D_GAS,), F32, kind="ExternalInput").ap()
    bge_d = nc.dram_tensor("bge", (D_GAS,), F32, kind="ExternalInput").ap()
    hb_d = nc.dram_tensor("hb", (D_GAS,), F32, kind="ExternalInput").ap()
    iden_d = nc.dram_tensor("iden", (128, 128), F32R, kind="ExternalInput").ap()
    ones_d = nc.dram_tensor("ones", (1, R), F32R, kind="ExternalInput").ap()
    out_d = nc.dram_tensor("out", (ROWS, D_OUT), F32, kind="ExternalOutput").ap()

    with tile.TileContext(nc) as tc:
        with (
            tc.tile_pool(name="const", bufs=1) as cpool,
            tc.tile_pool(name="state", bufs=2) as spool,
            tc.tile_pool(name="work", bufs=3) as wpool,
            tc.tile_pool(name="psum", bufs=1, space="PSUM") as ppool,
        ):
            # ---- constants ----
            wge_t = cpool.tile([128, 4, D_GAS], F32R, tag="wge")
            nc.sync.dma_start(wge_t, wge_d.rearrange("(ko ki) m -> ki ko m", ki=128))
            wdesc_t = cpool.tile([128, 2, D_GAS], F32R, tag="wdesc")
            nc.sync.dma_start(wdesc_t, wdesc_d.rearrange("(ko ki) m -> ki ko m", ki=128))
            wasc_t = cpool.tile([128, 2, D_GAS], F32R, tag="wasc")
            nc.sync.dma_start(wasc_t, wasc_d.rearrange("(ko ki) m -> ki ko m", ki=128))
            wfold_t = cpool.tile([128, 2, D_GAS], F32R, tag="wfold")
            nc.sync.dma_start(wfold_t, wfold_d.rearrange("(ko ki) m -> ki ko m", ki=128))
            w1g_t = cpool.tile([128, 2, D_GAS], F32R, tag="w1g")
            nc.sync.dma_start(w1g_t, w1g_d.rearrange("(ko ki) m -> ki ko m", ki=128))
            w2_t = cpool.tile([128, 2, D_OUT], F32R, tag="w2")
            nc.sync.dma_start(w2_t, w2_d.rearrange("(ko ki) n -> ki ko n", ki=128))
            nab_t = cpool.tile([1, D_GAS], F32R, tag="nab")
            nc.sync.dma_start(nab_t, nab_d)
            b2r_t = cpool.tile([1, D_OUT], F32R, tag="b2r")
            nc.sync.dma_start(b2r_t, b2r_d)
            be_t = cpool.tile([128, 8, 2], F32, tag="be")
            nc.sync.dma_start(be_t, be_d.rearrange("n (f k) -> k n f", k=128))
            ne9_t = cpool.tile([128, 2], F32, tag="ne9")
            nc.sync.dma_start(ne9_t, ne9_d.rearrange("(f k) -> k f", k=128))
            bge_t = cpool.tile([128, 2], F32, tag="bge")
            nc.sync.dma_start(bge_t, bge_d.rearrange("(f k) -> k f", k=128))
            hb_t = cpool.tile([128, 2], F32, tag="hb")
            nc.sync.dma_start(hb_t, hb_d.rearrange("(f k) -> k f", k=128))
            iden_t = cpool.tile([128, 128], F32R, tag="iden")
            nc.sync.dma_start(iden_t, iden_d)
            ones_t = cpool.tile([1, R], F32R, tag="ones")
            nc.sync.dma_start(ones_t, ones_d)

            for c in range(N_CHUNKS):
                par = c % 2
                r0 = c * R

                # ---- load x and transpose into [feature, row] layout ----
                xnat = []
                for rb in range(R // 128):
                    xn = wpool.tile([128, D_IN], F32R, tag="xnat", bufs=5)
                    nc.sync.dma_start(xn, x_d[r0 + rb * 128 : r0 + (rb + 1) * 128, :])
                    xnat.append(xn)
                xT = wpool.tile([128, 4, R], F32R, tag="xT", bufs=2)
                for k in range(4):
                    pt = ppool.tile([128, R], F32R, tag="pt", bufs=2)
                    for rb in range(R // 128):
                        nc.tensor.transpose(
                            pt[:, rb * 128 : (rb + 1) * 128],
                            xnat[rb][:, k * 128 : (k + 1) * 128],
                            iden_t,
                        )
                    nc.vector.tensor_copy(xT[:, k, :], pt)

                # ---- encoder: g0 = relu(x @ W_ge + b_ge), transposed ----
                p_enc = [ppool.tile([128, R], F32, tag=f"sweep{par}f{ft}", bufs=1, name=f"penc{c}_{ft}")
                         for ft in range(2)]
                for ft in range(2):
                    for k in range(4):
                        nc.tensor.matmul(
                            p_enc[ft],
                            lhsT=wge_t[:, k, ft * 128 : (ft + 1) * 128],
                            rhs=xT[:, k, :],
                            start=(k == 0),
                            stop=(k == 3),
                        )
                g0 = spool.tile([128, 2, R], F32R, tag="g0")
                for ft in range(2):
                    nc.scalar.activation(
                        g0[:, ft, :], p_enc[ft], AF.Relu,
                        bias=bge_t[:, ft : ft + 1],
                    )

                st = {}      # plate index -> current SBUF tile (e or g)
                S = None
                for sweep in range(2):
                    last = sweep == 1
                    # ---------- descending sweep ----------
                    pacc = wpool.tile([128, 2, R], F32, tag="pacc", bufs=2)
                    for n in range(N_PLATES, 0, -1):
                        df = wpool.tile([128, 2, R], F32R, tag="df", bufs=3)
                        g_prev = g0 if (sweep == 0 or n == 1) else st[n - 1]
                        if n == N_PLATES:
                            for ft in range(2):
                                nc.vector.tensor_scalar(
                                    df[:, ft, :], g_prev[:, ft, :],
                                    ne9_t[:, ft : ft + 1], None, OP.add,
                                )
                        else:
                            eng = (
                                nc.gpsimd
                                if (sweep, n) in DESC_DF_ON_GPSIMD
                                else nc.vector
                            )
                            eng.tensor_tensor(df, g_prev, st[n + 1], OP.subtract)
                        if last:
                            if n == N_PLATES:
                                S = spool.tile([128, 2, R], F32, tag="S")
                                nc.gpsimd.tensor_copy(S, df)
                            else:
                                nc.gpsimd.tensor_tensor(S, S, df, OP.add)
                        pd = [ppool.tile([128, R], F32, tag=f"sweep{par}f{ft}", bufs=1, name=f"pd{c}_{sweep}_{n}_{ft}")
                              for ft in range(2)]
                        for ft in range(2):
                            for k in range(2):
                                nc.tensor.matmul(
                                    pd[ft],
                                    lhsT=wdesc_t[:, k, ft * 128 : (ft + 1) * 128],
                                    rhs=df[:, k, :],
                                    start=(k == 0),
                                    stop=(k == 1),
                                )
                        e_new = spool.tile([128, 2, R], F32R, tag=f"st{n}")
                        for ft in range(2):
                            if n == N_PLATES:
                                nc.vector.tensor_copy(pacc[:, ft, :], pd[ft])
                            else:
                                nc.vector.tensor_tensor(
                                    pacc[:, ft, :], pacc[:, ft, :], pd[ft], OP.add
                                )
                            nc.scalar.activation(
                                e_new[:, ft, :], pacc[:, ft, :], AF.Sigmoid,
                                bias=be_t[:, n - 1, ft : ft + 1],
                            )
                        st[n] = e_new

                    # ---------- ascending sweep ----------
                    g_acc = wpool.tile([128, 2, R], F32, tag="gacc", bufs=2)
                    nc.vector.tensor_copy(g_acc, g0)
                    for n in range(1, N_PLATES + 1):
                        df = wpool.tile([128, 2, R], F32R, tag="df", bufs=3)
                        nc.vector.tensor_tensor(df, g_acc, st[n], OP.subtract)
                        pd = [ppool.tile([128, R], F32, tag=f"sweep{par}f{ft}", bufs=1, name=f"pa{c}_{sweep}_{n}_{ft}")
                              for ft in range(2)]
                        for ft in range(2):
                            for k in range(2):
                                nc.tensor.matmul(
                                    pd[ft],
                                    lhsT=wasc_t[:, k, ft * 128 : (ft + 1) * 128],
                                    rhs=df[:, k, :],
                                    start=(k == 0), stop=False,
                                )
                            nc.tensor.matmul(
                                pd[ft],
                                lhsT=nab_t[0:1, ft * 128 : (ft + 1) * 128],
                                rhs=ones_t[0:1, :],
                                start=False, stop=True,
                            )
                            nc.vector.tensor_tensor(
                                g_acc[:, ft, :], g_acc[:, ft, :], pd[ft], OP.add
                            )
                        if (not last and n <= N_PLATES - 1) or (last and n == N_PLATES):
                            g_sn = spool.tile([128, 2, R], F32R, tag=f"st{n}")
                            nc.scalar.copy(g_sn, g_acc)
                            st[n] = g_sn

                # ---------- head ----------
                S_r = wpool.tile([128, 2, R], F32R, tag="S_r", bufs=2)
                nc.vector.tensor_copy(S_r, S)
                g8 = st[N_PLATES]
                p_h = [ppool.tile([128, R], F32, tag=f"sweep{par}f{ft}", bufs=1, name=f"ph{c}_{ft}")
                       for ft in range(2)]
                for ft in range(2):
                    for k in range(2):
                        nc.tensor.matmul(
                            p_h[ft],
                            lhsT=w1g_t[:, k, ft * 128 : (ft + 1) * 128],
                            rhs=g8[:, k, :],
                            start=(k == 0), stop=False,
                        )
                    for k in range(2):
                        nc.tensor.matmul(
                            p_h[ft],
                            lhsT=wfold_t[:, k, ft * 128 : (ft + 1) * 128],
                            rhs=S_r[:, k, :],
                            start=False, stop=(k == 1),
                        )
                h = wpool.tile([128, 2, R], F32R, tag="h", bufs=2)
                for ft in range(2):
                    nc.scalar.activation(
                        h[:, ft, :], p_h[ft], AF.Relu,
                        bias=hb_t[:, ft : ft + 1],
                    )
                # out = h @ W2 + b2, with h as the stationary operand so the
                # result lands natural [row, feature]
                for rb in range(R // 128):
                    p_o = ppool.tile([128, D_OUT], F32, tag="po", bufs=1)
                    for n0, nw in ((0, 512), (512, 488)):
                        for ft in range(2):
                            nc.tensor.matmul(
                                p_o[:, n0 : n0 + nw],
                                lhsT=h[:, ft, rb * 128 : (rb + 1) * 128],
                                rhs=w2_t[:, ft, n0 : n0 + nw],
                                start=(ft == 0), stop=False,
                            )
                        nc.tensor.matmul(
                            p_o[:, n0 : n0 + nw],
                            lhsT=ones_t[0:1, 0:128],
                            rhs=b2r_t[0:1, n0 : n0 + nw],
                            start=False, stop=True,
                        )
                    stage = wpool.tile([128, D_OUT], F32, tag="stage", bufs=3)
                    nc.vector.tensor_copy(stage, p_o)
                    nc.sync.dma_start(
                        out_d[r0 + rb * 128 : r0 + (rb + 1) * 128, :], stage
                    )

    nc.compile()
    return nc


_NC_CACHE = {}


def kernel(**inputs):
    inp = {k: np.asarray(v) for k, v in inputs.items()}
    prep = _preprocess_weights(inp)
    x = np.ascontiguousarray(inp["x"], dtype=np.float32)

    if "nc" not in _NC_CACHE:
        _NC_CACHE["nc"] = build_nc()
    nc = _NC_CACHE["nc"]

    in_maps = []
    for c in range(N_CORES):
        m = {"x": x[c * ROWS : (c + 1) * ROWS]}
        m.update(prep)
        in_maps.append(m)
    res = bass_utils.run_bass_kernel_spmd(nc, in_maps, core_ids=list(range(N_CORES)))
    out = np.concatenate([res.results[c]["out"] for c in range(N_CORES)], axis=0)
    return out



# revision 4
# speedup vs baseline: 2.7477x; 2.7477x over previous
"""Trainium2 Bass kernel for nn_CounterFlowNetwork (v2).

Data-parallel over 8 NeuronCores (batch sharded).  Key structure vs v1:

 - Plate-major program order: the 4 row-chunks of a core are interleaved
   inside every plate step, so each engine pipeline (PE / ACT / DVE) sees
   independent work back-to-back and the PE never idles long enough for
   the HAM clock-gate to re-throttle it.
 - bf16 streams + weights everywhere (fp32 PSUM accumulation).  DVE
   element-wise ops on packed bf16 SBUF tiles run in 2x/4x mode; DMA
   traffic halves.  End-to-end quantization error simulated at ~6e-3,
   well under the 2e-2 gate.
 - x is transposed host-side, so no on-device transposes.
 - Descending sweep accumulates E_n = sum df@W_trabeq directly in PSUM
   across all 8 plates of a sweep (one long accumulation group); the
   sigmoid reads PSUM with the per-plate bias in the ACT bias slot.
 - Ascending sweep keeps g resident in PSUM (identity-matmul init with
   g0, then one accumulating matmul pair per plate).  The per-plate
   constant -alpha*b_tr is NOT injected on-device; instead the resulting
   deterministic drift eps_n (a linear recursion precomputed host-side)
   is cancelled in the bias slot of the ACT copies that materialize g.
 - S = sum of descending driving forces accumulates on GPSIMD (off the
   critical path); head output is computed transposed (W2 stationary)
   and the final b2 bias is added host-side after the gather.
"""

import numpy as np
import ml_dtypes

import concourse.bass as bass
import concourse.bacc as bacc
import concourse.mybir as mybir
import concourse.tile as tile
from concourse import bass_utils

B, D_IN, D_GAS, D_OUT = 16384, 512, 256, 1000
N_PLATES = 8
N_CORES = 8
ROWS = B // N_CORES          # rows per core
N_CHUNKS = 4
R = ROWS // N_CHUNKS         # rows per chunk
NB = 8                       # output feature blocks (1000 = 8 * 125)
BLK = D_OUT // NB
F32 = mybir.dt.float32
BF16 = mybir.dt.bfloat16
AF = mybir.ActivationFunctionType
OP = mybir.AluOpType
BF = ml_dtypes.bfloat16


def _preprocess_weights(inp):
    """Fold the plate linear algebra host-side (float64, cast to bf16/f32)."""
    f64 = np.float64
    W_tr = np.asarray(inp["W_tr"], f64)
    b_tr = np.asarray(inp["b_tr"], f64)
    W_ab = np.asarray(inp["W_ab"], f64)
    b_ab = np.asarray(inp["b_ab"], f64)
    W_eq = np.asarray(inp["W_eq"], f64)
    b_eq = np.asarray(inp["b_eq"], f64)
    W1 = np.asarray(inp["W1"], f64)
    b1 = np.asarray(inp["b1"], f64)
    alpha = float(np.asarray(inp["alpha"]))

    W_trp = alpha * W_tr
    c1 = alpha * b_tr                       # asc per-plate bias (row vec)
    W_trab = W_trp @ W_ab
    c2 = c1 @ W_ab + b_ab
    W_trabeq = W_trab @ W_eq                # desc folded weight
    c3 = c2 @ W_eq
    W1_g, W1_l = W1[:D_GAS], W1[D_GAS:]
    W_fold = W_trab @ W1_l
    wasc = -W_trp                           # asc folded weight

    be = np.stack([(b_eq + (9 - n) * c3) for n in range(1, 9)])   # (8, 256)
    e9 = 1.0 / (1.0 + np.exp(-b_eq))
    hb = b1 + 8.0 * (c2 @ W1_l)

    # eps_n: deterministic drift of the PSUM-resident g when the per-plate
    # -alpha*b_tr bias is omitted:  eps_n = eps_{n-1} @ (I + wasc) + c1.
    eps = np.zeros((9, D_GAS))
    for n in range(1, 9):
        eps[n] = eps[n - 1] @ (np.eye(D_GAS) + wasc) + c1

    def kt(w, kk):   # (K, M) -> [128, kk, M] k-tiled, bf16
        return np.ascontiguousarray(
            np.asarray(w, BF).reshape(kk, 128, -1).transpose(1, 0, 2))

    def pcol(v):     # (256,) -> [128, 2] partition-major f32
        return np.ascontiguousarray(
            np.asarray(v, np.float32).reshape(2, 128).T)

    return {
        "wge": kt(np.asarray(inp["W_ge"]), 4),
        "wdesc": kt(W_trabeq, 2),
        "wasc": kt(wasc, 2),
        "w1g": kt(W1_g, 2),
        "wfold": kt(W_fold, 2),
        "w2t": kt(np.asarray(inp["W2"]), 2),          # [128, 2, 1000]
        "iden": np.ascontiguousarray(np.eye(128, dtype=BF)),
        "bge": pcol(np.asarray(inp["b_ge"])),
        "hb": pcol(hb),
        "ne9": pcol(-e9),
        "be": np.ascontiguousarray(
            be.astype(np.float32).reshape(8, 2, 128).transpose(2, 0, 1)),
        "neps": np.ascontiguousarray(
            (-eps[1:]).astype(np.float32).reshape(8, 2, 128).transpose(2, 0, 1)),
    }


def build_nc():
    nc = bacc.Bacc("TRN2", target_bir_lowering=False, debug=False)

    xT_d = nc.dram_tensor("xT", (D_IN, ROWS), BF16, kind="ExternalInput").ap()
    wge_d = nc.dram_tensor("wge", (128, 4, D_GAS), BF16, kind="ExternalInput").ap()
    wdesc_d = nc.dram_tensor("wdesc", (128, 2, D_GAS), BF16, kind="ExternalInput").ap()
    wasc_d = nc.dram_tensor("wasc", (128, 2, D_GAS), BF16, kind="ExternalInput").ap()
    w1g_d = nc.dram_tensor("w1g", (128, 2, D_GAS), BF16, kind="ExternalInput").ap()
    wfold_d = nc.dram_tensor("wfold", (128, 2, D_GAS), BF16, kind="ExternalInput").ap()
    w2t_d = nc.dram_tensor("w2t", (128, 2, D_OUT), BF16, kind="ExternalInput").ap()
    iden_d = nc.dram_tensor("iden", (128, 128), BF16, kind="ExternalInput").ap()
    bge_d = nc.dram_tensor("bge", (128, 2), F32, kind="ExternalInput").ap()
    hb_d = nc.dram_tensor("hb", (128, 2), F32, kind="ExternalInput").ap()
    ne9_d = nc.dram_tensor("ne9", (128, 2), F32, kind="ExternalInput").ap()
    be_d = nc.dram_tensor("be", (128, 8, 2), F32, kind="ExternalInput").ap()
    neps_d = nc.dram_tensor("neps", (128, 8, 2), F32, kind="ExternalInput").ap()
    outT_d = nc.dram_tensor("outT", (D_OUT, ROWS), BF16, kind="ExternalOutput").ap()

    CH = range(N_CHUNKS)
    with tile.TileContext(nc) as tc:
        with (
            tc.tile_pool(name="const", bufs=1) as cpool,
            tc.tile_pool(name="state", bufs=1) as spool,
            tc.tile_pool(name="work", bufs=3) as wpool,
            tc.tile_pool(name="psum", bufs=1, space="PSUM") as ppool,
        ):
            # ---- constants ----
            wge_t = cpool.tile([128, 4, D_GAS], BF16, tag="wge")
            nc.sync.dma_start(wge_t, wge_d)
            wdesc_t = cpool.tile([128, 2, D_GAS], BF16, tag="wdesc")
            nc.sync.dma_start(wdesc_t, wdesc_d)
            wasc_t = cpool.tile([128, 2, D_GAS], BF16, tag="wasc")
            nc.sync.dma_start(wasc_t, wasc_d)
            w1g_t = cpool.tile([128, 2, D_GAS], BF16, tag="w1g")
            nc.sync.dma_start(w1g_t, w1g_d)
            wfold_t = cpool.tile([128, 2, D_GAS], BF16, tag="wfold")
            nc.sync.dma_start(wfold_t, wfold_d)
            w2t_t = cpool.tile([128, 2, D_OUT], BF16, tag="w2t")
            nc.sync.dma_start(w2t_t, w2t_d)
            iden_t = cpool.tile([128, 128], BF16, tag="iden")
            nc.sync.dma_start(iden_t, iden_d)
            bge_t = cpool.tile([128, 2], F32, tag="bge")
            nc.sync.dma_start(bge_t, bge_d)
            hb_t = cpool.tile([128, 2], F32, tag="hb")
            nc.sync.dma_start(hb_t, hb_d)
            ne9_t = cpool.tile([128, 2], F32, tag="ne9")
            nc.sync.dma_start(ne9_t, ne9_d)
            be_t = cpool.tile([128, 8, 2], F32, tag="be")
            nc.sync.dma_start(be_t, be_d)
            neps_t = cpool.tile([128, 8, 2], F32, tag="neps")
            nc.sync.dma_start(neps_t, neps_d)

            # ---- x load (already transposed host-side) ----
            xT = []
            for c in CH:
                xt = spool.tile([128, 4, R], BF16, tag=f"xT{c}")
                nc.sync.dma_start(
                    xt, xT_d[:, c * R:(c + 1) * R].rearrange(
                        "(ko ki) r -> ki ko r", ki=128))
                xT.append(xt)

            # per-chunk plate PSUM: [128, 2, R] f32 = 2 banks
            pp = [ppool.tile([128, 2, R], F32, tag=f"pp{c}", name=f"pp_enc{c}") for c in CH]

            # ---- encoder: g0 = relu(x @ W_ge + b_ge), transposed ----
            for ft in range(2):
                for k in range(4):
                    for c in CH:
                        nc.tensor.matmul(
                            pp[c][:, ft, :],
                            lhsT=wge_t[:, k, ft * 128:(ft + 1) * 128],
                            rhs=xT[c][:, k, :],
                            start=(k == 0), stop=(k == 3),
                        )
            g0 = []
            for c in CH:
                g = spool.tile([128, 2, R], BF16, tag=f"g0_{c}")
                for ft in range(2):
                    nc.scalar.activation(
                        g[:, ft, :], pp[c][:, ft, :], AF.Relu,
                        bias=bge_t[:, ft:ft + 1],
                    )
                g0.append(g)

            st = [{} for _ in CH]   # per chunk: plate idx -> SBUF tile
            S = [None] * N_CHUNKS
            g8 = [None] * N_CHUNKS

            for sweep in range(2):
                last = sweep == 1
                # ---------- descending sweep ----------
                ppd = [ppool.tile([128, 2, R], F32, tag=f"pp{c}", name=f"ppd{sweep}_{c}") for c in CH]
                for n in range(N_PLATES, 0, -1):
                    dfs = []
                    for c in CH:
                        df = wpool.tile([128, 2, R], BF16, tag=f"df{c}")
                        g_prev = g0[c] if (sweep == 0 or n == 1) else st[c][n - 1]
                        if n == N_PLATES:
                            for ft in range(2):
                                nc.vector.tensor_scalar(
                                    df[:, ft, :], g_prev[:, ft, :],
                                    ne9_t[:, ft:ft + 1], None, OP.add,
                                )
                        else:
                            nc.vector.tensor_tensor(
                                df, g_prev, st[c][n + 1], OP.subtract)
                        dfs.append(df)
                    for ft in range(2):
                        for k in range(2):
                            for c in CH:
                                nc.tensor.matmul(
                                    ppd[c][:, ft, :],
                                    lhsT=wdesc_t[:, k, ft * 128:(ft + 1) * 128],
                                    rhs=dfs[c][:, k, :],
                                    start=(n == N_PLATES and k == 0),
                                    stop=(n == 1 and k == 1),
                                    skip_group_check=True,
                                )
                    for c in CH:
                        e_new = spool.tile([128, 2, R], BF16, tag=f"st{n}_{c}")
                        for ft in range(2):
                            nc.scalar.activation(
                                e_new[:, ft, :], ppd[c][:, ft, :], AF.Sigmoid,
                                bias=be_t[:, n - 1, ft:ft + 1],
                            )
                        st[c][n] = e_new
                    if last:
                        for c in CH:
                            if n == N_PLATES:
                                S[c] = spool.tile([128, 2, R], BF16, tag=f"S{c}", name=f"S{sweep}_{c}")
                                nc.gpsimd.tensor_copy(S[c], dfs[c])
                            else:
                                nc.gpsimd.tensor_tensor(
                                    S[c], S[c], dfs[c], OP.add)

                # ---------- ascending sweep (g resident in PSUM) ----------
                ppa = [ppool.tile([128, 2, R], F32, tag=f"pp{c}", name=f"ppa{sweep}_{c}") for c in CH]
                for ft in range(2):
                    for c in CH:
                        nc.tensor.matmul(
                            ppa[c][:, ft, :],
                            lhsT=iden_t,
                            rhs=g0[c][:, ft, :],
                            start=True, stop=False,
                            skip_group_check=True,
                        )
                for n in range(1, N_PLATES + 1):
                    dfs = []
                    for c in CH:
                        df = wpool.tile([128, 2, R], BF16, tag=f"df{c}")
                        nc.vector.tensor_tensor(df, ppa[c], st[c][n], OP.subtract)
                        dfs.append(df)
                    for ft in range(2):
                        for k in range(2):
                            for c in CH:
                                nc.tensor.matmul(
                                    ppa[c][:, ft, :],
                                    lhsT=wasc_t[:, k, ft * 128:(ft + 1) * 128],
                                    rhs=dfs[c][:, k, :],
                                    start=False,
                                    stop=(n == N_PLATES and k == 1),
                                    skip_group_check=True,
                                )
                    if not last and n <= N_PLATES - 1:
                        for c in CH:
                            g_sn = spool.tile([128, 2, R], BF16, tag=f"st{n}_{c}")
                            for ft in range(2):
                                nc.scalar.activation(
                                    g_sn[:, ft, :], ppa[c][:, ft, :], AF.Identity,
                                    bias=neps_t[:, n - 1, ft:ft + 1],
                                )
                            st[c][n] = g_sn
                    elif last and n == N_PLATES:
                        for c in CH:
                            g8[c] = spool.tile([128, 2, R], BF16, tag=f"g8_{c}", name=f"g8_{c}")
                            for ft in range(2):
                                nc.scalar.activation(
                                    g8[c][:, ft, :], ppa[c][:, ft, :], AF.Identity,
                                    bias=neps_t[:, n - 1, ft:ft + 1],
                                )

            # ---------- head ----------
            pph = [ppool.tile([128, 2, R], F32, tag=f"pp{c}", name=f"pph{c}") for c in CH]
            for ft in range(2):
                for k in range(2):
                    for c in CH:
                        nc.tensor.matmul(
                            pph[c][:, ft, :],
                            lhsT=w1g_t[:, k, ft * 128:(ft + 1) * 128],
                            rhs=g8[c][:, k, :],
                            start=(k == 0), stop=False,
                        )
                for k in range(2):
                    for c in CH:
                        nc.tensor.matmul(
                            pph[c][:, ft, :],
                            lhsT=wfold_t[:, k, ft * 128:(ft + 1) * 128],
                            rhs=S[c][:, k, :],
                            start=False, stop=(k == 1),
                        )
            hs = []
            for c in CH:
                h = spool.tile([128, 2, R], BF16, tag=f"h{c}")
                for ft in range(2):
                    nc.scalar.activation(
                        h[:, ft, :], pph[c][:, ft, :], AF.Relu,
                        bias=hb_t[:, ft:ft + 1],
                    )
                hs.append(h)
            # out^T blocks: po[b] = W2[:, b*125:(b+1)*125]^T @ h
            for b in range(NB):
                pos = []
                for c in CH:
                    po = ppool.tile([128, 2, R], F32, tag=f"pp{c}", name=f"po{b}_{c}")
                    for k in range(2):
                        nc.tensor.matmul(
                            po[:BLK, b % 2, :],
                            lhsT=w2t_t[:, k, b * BLK:(b + 1) * BLK],
                            rhs=hs[c][:, k, :],
                            start=(k == 0), stop=(k == 1),
                        )
                    pos.append(po)
                for c in CH:
                    stg = wpool.tile([128, R], BF16, tag=f"stg{c}", bufs=2)
                    eng = nc.scalar if (b + c) % 2 == 0 else nc.vector
                    if eng is nc.scalar:
                        nc.scalar.copy(stg[:BLK, :], pos[c][:BLK, b % 2, :])
                    else:
                        nc.vector.tensor_copy(stg[:BLK, :], pos[c][:BLK, b % 2, :])
                    nc.sync.dma_start(
                        outT_d[b * BLK:(b + 1) * BLK, c * R:(c + 1) * R],
                        stg[:BLK, :],
                    )

    nc.compile()
    return nc


def _shard_inputs(inp):
    prep = _preprocess_weights(inp)
    xT = np.ascontiguousarray(
        np.asarray(inp["x"], np.float32).astype(BF).T)   # (512, 16384)
    in_maps = []
    for c in range(N_CORES):
        m = {"xT": np.ascontiguousarray(xT[:, c * ROWS:(c + 1) * ROWS])}
        m.update(prep)
        in_maps.append(m)
    return in_maps


def _gather_output(inp, res):
    outT = np.concatenate(
        [res.results[c]["outT"] for c in range(N_CORES)], axis=1)
    out = outT.T.astype(np.float32) + np.asarray(inp["b2"], np.float32)
    return out


_NC_CACHE = {}


def kernel(**inputs):
    inp = {k: np.asarray(v) for k, v in inputs.items()}
    if "nc" not in _NC_CACHE:
        _NC_CACHE["nc"] = build_nc()
    nc = _NC_CACHE["nc"]
    in_maps = _shard_inputs(inp)
    res = bass_utils.run_bass_kernel_spmd(nc, in_maps, core_ids=list(range(N_CORES)))
    return _gather_output(inp, res)


# revision 5
# speedup vs baseline: 3.3922x; 1.2345x over previous
"""Trainium2 Bass kernel for nn_CounterFlowNetwork (v3).

Data-parallel over 8 NeuronCores (batch sharded).  Structure:

 - bf16 streams + weights (fp32 PSUM accumulation); x transposed
   host-side; head computed transposed (W2 stationary); final b2 bias
   added host-side after the gather.
 - Descending sweep accumulates E_n in PSUM across all 8 plates (one
   long accumulation group); sigmoid reads PSUM with the per-plate bias
   in the ACT bias slot.
 - Ascending sweep keeps g resident in PSUM (identity-matmul init with
   g0); the omitted per-plate -alpha*b_tr bias produces a deterministic
   drift eps_n that is cancelled in the bias slot of the ACT copies
   which materialize g.
 - The 4 row-chunks are processed as TWO groups of 2, emitted as
   interleaved step lists with a half-sweep phase offset: when one
   group hits a serial sweep transition, the other group's matmuls keep
   the PE busy so the HAM clock-gate stays at full rate.
 - S (sum of descending driving forces) accumulates on DVE in bf16;
   GPSIMD is left idle because concurrent GPSIMD traffic halves DVE
   throughput (shared SBUF ports).
"""

import numpy as np
import ml_dtypes

import concourse.bass as bass
import concourse.bacc as bacc
import concourse.mybir as mybir
import concourse.tile as tile
from concourse import bass_utils

B, D_IN, D_GAS, D_OUT = 16384, 512, 256, 1000
N_PLATES = 8
N_CORES = 8
ROWS = B // N_CORES          # rows per core
N_CHUNKS = 4
R = ROWS // N_CHUNKS         # rows per chunk
NB = 8                       # output feature blocks (1000 = 8 * 125)
BLK = D_OUT // NB
STAGGER = 9                  # step offset between the two chunk groups
F32 = mybir.dt.float32
BF16 = mybir.dt.bfloat16
AF = mybir.ActivationFunctionType
OP = mybir.AluOpType
BF = ml_dtypes.bfloat16


def _preprocess_weights(inp):
    """Fold the plate linear algebra host-side (float64, cast to bf16/f32)."""
    f64 = np.float64
    W_tr = np.asarray(inp["W_tr"], f64)
    b_tr = np.asarray(inp["b_tr"], f64)
    W_ab = np.asarray(inp["W_ab"], f64)
    b_ab = np.asarray(inp["b_ab"], f64)
    W_eq = np.asarray(inp["W_eq"], f64)
    b_eq = np.asarray(inp["b_eq"], f64)
    W1 = np.asarray(inp["W1"], f64)
    b1 = np.asarray(inp["b1"], f64)
    alpha = float(np.asarray(inp["alpha"]))

    W_trp = alpha * W_tr
    c1 = alpha * b_tr                       # asc per-plate bias (row vec)
    W_trab = W_trp @ W_ab
    c2 = c1 @ W_ab + b_ab
    W_trabeq = W_trab @ W_eq                # desc folded weight
    c3 = c2 @ W_eq
    W1_g, W1_l = W1[:D_GAS], W1[D_GAS:]
    W_fold = W_trab @ W1_l
    wasc = -W_trp                           # asc folded weight

    be = np.stack([(b_eq + (9 - n) * c3) for n in range(1, 9)])   # (8, 256)
    e9 = 1.0 / (1.0 + np.exp(-b_eq))
    hb = b1 + 8.0 * (c2 @ W1_l)

    # eps_n: deterministic drift of the PSUM-resident g when the per-plate
    # -alpha*b_tr bias is omitted:  eps_n = eps_{n-1} @ (I + wasc) + c1.
    eps = np.zeros((9, D_GAS))
    for n in range(1, 9):
        eps[n] = eps[n - 1] @ (np.eye(D_GAS) + wasc) + c1

    def kt(w, kk):   # (K, M) -> [128, kk, M] k-tiled, bf16
        return np.ascontiguousarray(
            np.asarray(w, BF).reshape(kk, 128, -1).transpose(1, 0, 2))

    def pcol(v):     # (256,) -> [128, 2] partition-major f32
        return np.ascontiguousarray(
            np.asarray(v, np.float32).reshape(2, 128).T)

    return {
        "wge": kt(np.asarray(inp["W_ge"]), 4),
        "wdesc": kt(W_trabeq, 2),
        "wasc": kt(wasc, 2),
        "w1g": kt(W1_g, 2),
        "wfold": kt(W_fold, 2),
        "w2t": kt(np.asarray(inp["W2"]), 2),          # [128, 2, 1000]
        "iden": np.ascontiguousarray(np.eye(128, dtype=BF)),
        "bge": pcol(np.asarray(inp["b_ge"])),
        "hb": pcol(hb),
        "ne9": pcol(-e9),
        "be": np.ascontiguousarray(
            be.astype(np.float32).reshape(8, 2, 128).transpose(2, 0, 1)),
        "neps": np.ascontiguousarray(
            (-eps[1:]).astype(np.float32).reshape(8, 2, 128).transpose(2, 0, 1)),
    }


def build_nc():
    nc = bacc.Bacc("TRN2", target_bir_lowering=False, debug=False)

    xT_d = nc.dram_tensor("xT", (D_IN, ROWS), BF16, kind="ExternalInput").ap()
    wge_d = nc.dram_tensor("wge", (128, 4, D_GAS), BF16, kind="ExternalInput").ap()
    wdesc_d = nc.dram_tensor("wdesc", (128, 2, D_GAS), BF16, kind="ExternalInput").ap()
    wasc_d = nc.dram_tensor("wasc", (128, 2, D_GAS), BF16, kind="ExternalInput").ap()
    w1g_d = nc.dram_tensor("w1g", (128, 2, D_GAS), BF16, kind="ExternalInput").ap()
    wfold_d = nc.dram_tensor("wfold", (128, 2, D_GAS), BF16, kind="ExternalInput").ap()
    w2t_d = nc.dram_tensor("w2t", (128, 2, D_OUT), BF16, kind="ExternalInput").ap()
    iden_d = nc.dram_tensor("iden", (128, 128), BF16, kind="ExternalInput").ap()
    bge_d = nc.dram_tensor("bge", (128, 2), F32, kind="ExternalInput").ap()
    hb_d = nc.dram_tensor("hb", (128, 2), F32, kind="ExternalInput").ap()
    ne9_d = nc.dram_tensor("ne9", (128, 2), F32, kind="ExternalInput").ap()
    be_d = nc.dram_tensor("be", (128, 8, 2), F32, kind="ExternalInput").ap()
    neps_d = nc.dram_tensor("neps", (128, 8, 2), F32, kind="ExternalInput").ap()
    outT_d = nc.dram_tensor("outT", (D_OUT, ROWS), BF16, kind="ExternalOutput").ap()

    with tile.TileContext(nc) as tc:
        with (
            tc.tile_pool(name="const", bufs=1) as cpool,
            tc.tile_pool(name="state", bufs=1) as spool,
            tc.tile_pool(name="work", bufs=3) as wpool,
            tc.tile_pool(name="psum", bufs=1, space="PSUM") as ppool,
        ):
            # ---- constants ----
            wge_t = cpool.tile([128, 4, D_GAS], BF16, tag="wge")
            nc.sync.dma_start(wge_t, wge_d)
            wdesc_t = cpool.tile([128, 2, D_GAS], BF16, tag="wdesc")
            nc.sync.dma_start(wdesc_t, wdesc_d)
            wasc_t = cpool.tile([128, 2, D_GAS], BF16, tag="wasc")
            nc.sync.dma_start(wasc_t, wasc_d)
            w1g_t = cpool.tile([128, 2, D_GAS], BF16, tag="w1g")
            nc.sync.dma_start(w1g_t, w1g_d)
            wfold_t = cpool.tile([128, 2, D_GAS], BF16, tag="wfold")
            nc.sync.dma_start(wfold_t, wfold_d)
            w2t_t = cpool.tile([128, 2, D_OUT], BF16, tag="w2t")
            nc.sync.dma_start(w2t_t, w2t_d)
            iden_t = cpool.tile([128, 128], BF16, tag="iden")
            nc.sync.dma_start(iden_t, iden_d)
            bge_t = cpool.tile([128, 2], F32, tag="bge")
            nc.sync.dma_start(bge_t, bge_d)
            hb_t = cpool.tile([128, 2], F32, tag="hb")
            nc.sync.dma_start(hb_t, hb_d)
            ne9_t = cpool.tile([128, 2], F32, tag="ne9")
            nc.sync.dma_start(ne9_t, ne9_d)
            be_t = cpool.tile([128, 8, 2], F32, tag="be")
            nc.sync.dma_start(be_t, be_d)
            neps_t = cpool.tile([128, 8, 2], F32, tag="neps")
            nc.sync.dma_start(neps_t, neps_d)

            # ---- x load (already transposed host-side), all chunks ----
            xT = [None] * N_CHUNKS
            for c in range(N_CHUNKS):
                xt = spool.tile([128, 4, R], BF16, tag=f"xT{c}", name=f"xT{c}")
                nc.sync.dma_start(
                    xt, xT_d[:, c * R:(c + 1) * R].rearrange(
                        "(ko ki) r -> ki ko r", ki=128))
                xT[c] = xt

            # shared state across step closures
            g0 = [None] * N_CHUNKS
            st = [{} for _ in range(N_CHUNKS)]
            S = [None] * N_CHUNKS
            g8 = [None] * N_CHUNKS
            hs = [None] * N_CHUNKS
            pband = [None] * N_CHUNKS      # current PSUM tile per chunk

            def mm_block(chs, pget, lhsT_t, rhs_list, ft_r, k_r, start_fn, stop_fn):
                for ft in ft_r:
                    for k in k_r:
                        for c in chs:
                            nc.tensor.matmul(
                                pget(c)[:, ft, :],
                                lhsT=lhsT_t[:, k, ft * 128:(ft + 1) * 128],
                                rhs=rhs_list[c][:, k, :],
                                start=start_fn(ft, k),
                                stop=stop_fn(ft, k),
                                skip_group_check=True,
                            )

            def group_steps(chs, gi):
                steps = []

                def s_enc():
                    for c in chs:
                        pband[c] = ppool.tile([128, 2, R], F32, tag=f"pp{c}",
                                              name=f"pp_enc{c}")
                    for ft in range(2):
                        for k in range(4):
                            for c in chs:
                                nc.tensor.matmul(
                                    pband[c][:, ft, :],
                                    lhsT=wge_t[:, k, ft * 128:(ft + 1) * 128],
                                    rhs=xT[c][:, k, :],
                                    start=(k == 0), stop=(k == 3),
                                )
                    for c in chs:
                        g = spool.tile([128, 2, R], BF16, tag=f"g0_{c}",
                                       name=f"g0_{c}")
                        for ft in range(2):
                            nc.scalar.activation(
                                g[:, ft, :], pband[c][:, ft, :], AF.Relu,
                                bias=bge_t[:, ft:ft + 1],
                            )
                        g0[c] = g
                steps.append(s_enc)

                for sweep in range(2):
                    last = sweep == 1

                    def make_desc(n, sweep=sweep, last=last):
                        def s_desc():
                            if n == N_PLATES:
                                for c in chs:
                                    pband[c] = ppool.tile(
                                        [128, 2, R], F32, tag=f"pp{c}",
                                        name=f"ppd{sweep}_{c}")
                            dfs = {}
                            for c in chs:
                                df = wpool.tile([128, 2, R], BF16,
                                                tag=f"df{c}", name=f"dfd{sweep}_{n}_{c}")
                                g_prev = g0[c] if (sweep == 0 or n == 1) else st[c][n - 1]
                                if n == N_PLATES:
                                    for ft in range(2):
                                        nc.vector.tensor_scalar(
                                            df[:, ft, :], g_prev[:, ft, :],
                                            ne9_t[:, ft:ft + 1], None, OP.add,
                                        )
                                else:
                                    nc.vector.tensor_tensor(
                                        df, g_prev, st[c][n + 1], OP.subtract)
                                dfs[c] = df
                            mm_block(
                                chs, lambda c: pband[c], wdesc_t, dfs,
                                range(2), range(2),
                                lambda ft, k: (n == N_PLATES and k == 0),
                                lambda ft, k: (n == 1 and k == 1),
                            )
                            for c in chs:
                                e_new = spool.tile([128, 2, R], BF16,
                                                   tag=f"st{n}_{c}",
                                                   name=f"e{sweep}_{n}_{c}")
                                for ft in range(2):
                                    nc.scalar.activation(
                                        e_new[:, ft, :], pband[c][:, ft, :],
                                        AF.Sigmoid,
                                        bias=be_t[:, n - 1, ft:ft + 1],
                                    )
                                st[c][n] = e_new
                            if last:
                                for c in chs:
                                    if n == N_PLATES:
                                        S[c] = spool.tile(
                                            [128, 2, R], BF16, tag=f"S{c}",
                                            name=f"S_{c}")
                                        nc.vector.tensor_copy(S[c], dfs[c])
                                    else:
                                        nc.vector.tensor_tensor(
                                            S[c], S[c], dfs[c], OP.add)
                        return s_desc

                    for n in range(N_PLATES, 0, -1):
                        steps.append(make_desc(n))

                    def make_asc(n, sweep=sweep, last=last):
                        def s_asc():
                            if n == 1:
                                for c in chs:
                                    pband[c] = ppool.tile(
                                        [128, 2, R], F32, tag=f"pp{c}",
                                        name=f"ppa{sweep}_{c}")
                                for ft in range(2):
                                    for c in chs:
                                        nc.tensor.matmul(
                                            pband[c][:, ft, :],
                                            lhsT=iden_t,
                                            rhs=g0[c][:, ft, :],
                                            start=True, stop=False,
                                            skip_group_check=True,
                                        )
                            dfs = {}
                            for c in chs:
                                df = wpool.tile([128, 2, R], BF16,
                                                tag=f"df{c}", name=f"dfa{sweep}_{n}_{c}")
                                nc.vector.tensor_tensor(
                                    df, pband[c], st[c][n], OP.subtract)
                                dfs[c] = df
                            mm_block(
                                chs, lambda c: pband[c], wasc_t, dfs,
                                range(2), range(2),
                                lambda ft, k: False,
                                lambda ft, k: (n == N_PLATES and k == 1),
                            )
                            if not last and n <= N_PLATES - 1:
                                for c in chs:
                                    g_sn = spool.tile([128, 2, R], BF16,
                                                      tag=f"st{n}_{c}",
                                                      name=f"gs{sweep}_{n}_{c}")
                                    for ft in range(2):
                                        nc.scalar.activation(
                                            g_sn[:, ft, :], pband[c][:, ft, :],
                                            AF.Identity,
                                            bias=neps_t[:, n - 1, ft:ft + 1],
                                        )
                                    st[c][n] = g_sn
                            elif last and n == N_PLATES:
                                for c in chs:
                                    g8[c] = spool.tile([128, 2, R], BF16,
                                                       tag=f"g8_{c}",
                                                       name=f"g8_{c}")
                                    for ft in range(2):
                                        nc.scalar.activation(
                                            g8[c][:, ft, :], pband[c][:, ft, :],
                                            AF.Identity,
                                            bias=neps_t[:, n - 1, ft:ft + 1],
                                        )
                        return s_asc

                    for n in range(1, N_PLATES + 1):
                        steps.append(make_asc(n))

                def s_head():
                    for c in chs:
                        pband[c] = ppool.tile([128, 2, R], F32, tag=f"pp{c}",
                                              name=f"pph{c}")
                    mm_block(chs, lambda c: pband[c], w1g_t, g8,
                             range(2), range(2),
                             lambda ft, k: (k == 0), lambda ft, k: False)
                    mm_block(chs, lambda c: pband[c], wfold_t, S,
                             range(2), range(2),
                             lambda ft, k: False, lambda ft, k: (k == 1))
                    for c in chs:
                        h = spool.tile([128, 2, R], BF16, tag=f"h{c}",
                                       name=f"h{c}")
                        for ft in range(2):
                            nc.scalar.activation(
                                h[:, ft, :], pband[c][:, ft, :], AF.Relu,
                                bias=hb_t[:, ft:ft + 1],
                            )
                        hs[c] = h
                steps.append(s_head)

                def make_po(b):
                    def s_po():
                        pos = {}
                        for c in chs:
                            po = ppool.tile([128, 2, R], F32, tag=f"pp{c}",
                                            name=f"po{b}_{c}")
                            for k in range(2):
                                nc.tensor.matmul(
                                    po[:BLK, b % 2, :],
                                    lhsT=w2t_t[:, k, b * BLK:(b + 1) * BLK],
                                    rhs=hs[c][:, k, :],
                                    start=(k == 0), stop=(k == 1),
                                )
                            pos[c] = po
                        for c in chs:
                            stg = wpool.tile([128, R], BF16, tag=f"stg{c}",
                                             bufs=2, name=f"stg{b}_{c}")
                            if (b + c) % 2 == 0:
                                nc.scalar.copy(stg[:BLK, :], pos[c][:BLK, b % 2, :])
                            else:
                                nc.vector.tensor_copy(
                                    stg[:BLK, :], pos[c][:BLK, b % 2, :])
                            nc.sync.dma_start(
                                outT_d[b * BLK:(b + 1) * BLK,
                                       c * R:(c + 1) * R],
                                stg[:BLK, :],
                            )
                    return s_po

                for b in range(NB):
                    steps.append(make_po(b))
                return steps

            A = group_steps((0, 1), 0)
            Bs = group_steps((2, 3), 1)
            for i in range(len(A) + STAGGER):
                if i < len(A):
                    A[i]()
                j = i - STAGGER
                if 0 <= j < len(Bs):
                    Bs[j]()

    nc.compile()
    return nc


def _shard_inputs(inp):
    prep = _preprocess_weights(inp)
    xT = np.ascontiguousarray(
        np.asarray(inp["x"], np.float32).astype(BF).T)   # (512, 16384)
    in_maps = []
    for c in range(N_CORES):
        m = {"xT": np.ascontiguousarray(xT[:, c * ROWS:(c + 1) * ROWS])}
        m.update(prep)
        in_maps.append(m)
    return in_maps


def _gather_output(inp, res):
    outT = np.concatenate(
        [res.results[c]["outT"] for c in range(N_CORES)], axis=1)
    out = outT.T.astype(np.float32) + np.asarray(inp["b2"], np.float32)
    return out


_NC_CACHE = {}


def kernel(**inputs):
    inp = {k: np.asarray(v) for k, v in inputs.items()}
    if "nc" not in _NC_CACHE:
        _NC_CACHE["nc"] = build_nc()
    nc = _NC_CACHE["nc"]
    in_maps = _shard_inputs(inp)
    res = bass_utils.run_bass_kernel_spmd(nc, in_maps, core_ids=list(range(N_CORES)))
    return _gather_output(inp, res)


# revision 6
# speedup vs baseline: 3.4760x; 1.0247x over previous
"""Trainium2 Bass kernel for nn_CounterFlowNetwork (v3).

Data-parallel over 8 NeuronCores (batch sharded).  Structure:

 - bf16 streams + weights (fp32 PSUM accumulation); x transposed
   host-side; head computed transposed (W2 stationary); final b2 bias
   added host-side after the gather.
 - Descending sweep accumulates E_n in PSUM across all 8 plates (one
   long accumulation group); sigmoid reads PSUM with the per-plate bias
   in the ACT bias slot.
 - Ascending sweep keeps g resident in PSUM (identity-matmul init with
   g0); the omitted per-plate -alpha*b_tr bias produces a deterministic
   drift eps_n that is cancelled in the bias slot of the ACT copies
   which materialize g.
 - The 4 row-chunks are processed as TWO groups of 2, emitted as
   interleaved step lists with a half-sweep phase offset: when one
   group hits a serial sweep transition, the other group's matmuls keep
   the PE busy so the HAM clock-gate stays at full rate.
 - S (sum of descending driving forces) accumulates on DVE in bf16;
   GPSIMD is left idle because concurrent GPSIMD traffic halves DVE
   throughput (shared SBUF ports).
"""

import numpy as np
import ml_dtypes

import concourse.bass as bass
import concourse.bacc as bacc
import concourse.mybir as mybir
import concourse.tile as tile
from concourse import bass_utils

B, D_IN, D_GAS, D_OUT = 16384, 512, 256, 1000
N_PLATES = 8
N_CORES = 8
ROWS = B // N_CORES          # rows per core
N_CHUNKS = 4
R = ROWS // N_CHUNKS         # rows per chunk
NB = 8                       # output feature blocks (1000 = 8 * 125)
BLK = D_OUT // NB
STAGGER = 9                  # step offset between the two chunk groups
F32 = mybir.dt.float32
BF16 = mybir.dt.bfloat16
AF = mybir.ActivationFunctionType
OP = mybir.AluOpType
BF = ml_dtypes.bfloat16


def _preprocess_weights(inp):
    """Fold the plate linear algebra host-side (float64, cast to bf16/f32)."""
    f64 = np.float64
    W_tr = np.asarray(inp["W_tr"], f64)
    b_tr = np.asarray(inp["b_tr"], f64)
    W_ab = np.asarray(inp["W_ab"], f64)
    b_ab = np.asarray(inp["b_ab"], f64)
    W_eq = np.asarray(inp["W_eq"], f64)
    b_eq = np.asarray(inp["b_eq"], f64)
    W1 = np.asarray(inp["W1"], f64)
    b1 = np.asarray(inp["b1"], f64)
    alpha = float(np.asarray(inp["alpha"]))

    W_trp = alpha * W_tr
    c1 = alpha * b_tr                       # asc per-plate bias (row vec)
    W_trab = W_trp @ W_ab
    c2 = c1 @ W_ab + b_ab
    W_trabeq = W_trab @ W_eq                # desc folded weight
    c3 = c2 @ W_eq
    W1_g, W1_l = W1[:D_GAS], W1[D_GAS:]
    W_fold = W_trab @ W1_l
    wasc = -W_trp                           # asc folded weight

    be = np.stack([(b_eq + (9 - n) * c3) for n in range(1, 9)])   # (8, 256)
    e9 = 1.0 / (1.0 + np.exp(-b_eq))
    hb = b1 + 8.0 * (c2 @ W1_l)

    # eps_n: deterministic drift of the PSUM-resident g when the per-plate
    # -alpha*b_tr bias is omitted.  df' is always computed from the
    # bias-corrected materialized g tiles, so the drift is exactly n*c1.
    eps = np.stack([n * c1 for n in range(9)])

    def kt(w, kk):   # (K, M) -> [128, kk, M] k-tiled, bf16
        return np.ascontiguousarray(
            np.asarray(w, BF).reshape(kk, 128, -1).transpose(1, 0, 2))

    def pcol(v):     # (256,) -> [128, 2] partition-major f32
        return np.ascontiguousarray(
            np.asarray(v, np.float32).reshape(2, 128).T)

    return {
        "wge": kt(np.asarray(inp["W_ge"]), 4),
        "wdesc": kt(W_trabeq, 2),
        "wasc": kt(wasc, 2),
        "w1g": kt(W1_g, 2),
        "wfold": kt(W_fold, 2),
        "w2t": kt(np.asarray(inp["W2"]), 2),          # [128, 2, 1000]
        "iden": np.ascontiguousarray(np.eye(128, dtype=BF)),
        "bge": pcol(np.asarray(inp["b_ge"])),
        "hb": pcol(hb),
        "ne9": pcol(-e9),
        "be": np.ascontiguousarray(
            be.astype(np.float32).reshape(8, 2, 128).transpose(2, 0, 1)),
        "neps": np.ascontiguousarray(
            (-eps[1:]).astype(np.float32).reshape(8, 2, 128).transpose(2, 0, 1)),
    }


def build_nc():
    nc = bacc.Bacc("TRN2", target_bir_lowering=False, debug=False)

    xT_d = nc.dram_tensor("xT", (D_IN, ROWS), BF16, kind="ExternalInput").ap()
    wge_d = nc.dram_tensor("wge", (128, 4, D_GAS), BF16, kind="ExternalInput").ap()
    wdesc_d = nc.dram_tensor("wdesc", (128, 2, D_GAS), BF16, kind="ExternalInput").ap()
    wasc_d = nc.dram_tensor("wasc", (128, 2, D_GAS), BF16, kind="ExternalInput").ap()
    w1g_d = nc.dram_tensor("w1g", (128, 2, D_GAS), BF16, kind="ExternalInput").ap()
    wfold_d = nc.dram_tensor("wfold", (128, 2, D_GAS), BF16, kind="ExternalInput").ap()
    w2t_d = nc.dram_tensor("w2t", (128, 2, D_OUT), BF16, kind="ExternalInput").ap()
    iden_d = nc.dram_tensor("iden", (128, 128), BF16, kind="ExternalInput").ap()
    bge_d = nc.dram_tensor("bge", (128, 2), F32, kind="ExternalInput").ap()
    hb_d = nc.dram_tensor("hb", (128, 2), F32, kind="ExternalInput").ap()
    ne9_d = nc.dram_tensor("ne9", (128, 2), F32, kind="ExternalInput").ap()
    be_d = nc.dram_tensor("be", (128, 8, 2), F32, kind="ExternalInput").ap()
    neps_d = nc.dram_tensor("neps", (128, 8, 2), F32, kind="ExternalInput").ap()
    outT_d = nc.dram_tensor("outT", (D_OUT, ROWS), BF16, kind="ExternalOutput").ap()

    with tile.TileContext(nc) as tc:
        with (
            tc.tile_pool(name="const", bufs=1) as cpool,
            tc.tile_pool(name="state", bufs=1) as spool,
            tc.tile_pool(name="work", bufs=3) as wpool,
            tc.tile_pool(name="psum", bufs=1, space="PSUM") as ppool,
        ):
            # ---- constants + x: DMA-ordered so group A's encoder and the
            # first descending plates can start as early as possible ----
            xT = [None] * N_CHUNKS

            def load_x(c):
                xt = spool.tile([128, 4, R], BF16, tag=f"xT{c}", name=f"xT{c}")
                nc.sync.dma_start(
                    xt, xT_d[:, c * R:(c + 1) * R].rearrange(
                        "(ko ki) r -> ki ko r", ki=128))
                xT[c] = xt

            wge_t = cpool.tile([128, 4, D_GAS], BF16, tag="wge")
            nc.sync.dma_start(wge_t, wge_d)
            bge_t = cpool.tile([128, 2], F32, tag="bge")
            nc.sync.dma_start(bge_t, bge_d)
            load_x(0)
            load_x(1)
            wdesc_t = cpool.tile([128, 2, D_GAS], BF16, tag="wdesc")
            nc.sync.dma_start(wdesc_t, wdesc_d)
            be_t = cpool.tile([128, 8, 2], F32, tag="be")
            nc.sync.dma_start(be_t, be_d)
            ne9_t = cpool.tile([128, 2], F32, tag="ne9")
            nc.sync.dma_start(ne9_t, ne9_d)
            load_x(2)
            load_x(3)
            wasc_t = cpool.tile([128, 2, D_GAS], BF16, tag="wasc")
            nc.sync.dma_start(wasc_t, wasc_d)
            iden_t = cpool.tile([128, 128], BF16, tag="iden")
            nc.sync.dma_start(iden_t, iden_d)
            neps_t = cpool.tile([128, 8, 2], F32, tag="neps")
            nc.sync.dma_start(neps_t, neps_d)
            w1g_t = cpool.tile([128, 2, D_GAS], BF16, tag="w1g")
            nc.sync.dma_start(w1g_t, w1g_d)
            wfold_t = cpool.tile([128, 2, D_GAS], BF16, tag="wfold")
            nc.sync.dma_start(wfold_t, wfold_d)
            hb_t = cpool.tile([128, 2], F32, tag="hb")
            nc.sync.dma_start(hb_t, hb_d)
            w2t_t = cpool.tile([128, 2, D_OUT], BF16, tag="w2t")
            nc.sync.dma_start(w2t_t, w2t_d)

            # shared state across step closures
            g0 = [None] * N_CHUNKS
            st = [{} for _ in range(N_CHUNKS)]
            S = [None] * N_CHUNKS
            g8 = [None] * N_CHUNKS
            hs = [None] * N_CHUNKS
            pband = [None] * N_CHUNKS      # current PSUM tile per chunk

            def mm_block(chs, pget, lhsT_t, rhs_list, ft_r, k_r, start_fn, stop_fn):
                for ft in ft_r:
                    for k in k_r:
                        for c in chs:
                            nc.tensor.matmul(
                                pget(c)[:, ft, :],
                                lhsT=lhsT_t[:, k, ft * 128:(ft + 1) * 128],
                                rhs=rhs_list[c][:, k, :],
                                start=start_fn(ft, k),
                                stop=stop_fn(ft, k),
                                skip_group_check=True,
                            )

            def group_steps(chs, gi):
                steps = []

                def s_enc():
                    for c in chs:
                        pband[c] = ppool.tile([128, 2, R], F32, tag=f"pp{c}",
                                              name=f"pp_enc{c}")
                    for ft in range(2):
                        for k in range(4):
                            for c in chs:
                                nc.tensor.matmul(
                                    pband[c][:, ft, :],
                                    lhsT=wge_t[:, k, ft * 128:(ft + 1) * 128],
                                    rhs=xT[c][:, k, :],
                                    start=(k == 0), stop=(k == 3),
                                )
                    for c in chs:
                        g = spool.tile([128, 2, R], BF16, tag=f"g0_{c}",
                                       name=f"g0_{c}")
                        for ft in range(2):
                            nc.scalar.activation(
                                g[:, ft, :], pband[c][:, ft, :], AF.Relu,
                                bias=bge_t[:, ft:ft + 1],
                            )
                        g0[c] = g
                steps.append(s_enc)

                for sweep in range(2):
                    last = sweep == 1

                    def make_desc(n, sweep=sweep, last=last):
                        def s_desc():
                            if n == N_PLATES:
                                for c in chs:
                                    pband[c] = ppool.tile(
                                        [128, 2, R], F32, tag=f"pp{c}",
                                        name=f"ppd{sweep}_{c}")
                            dfs = {}
                            for c in chs:
                                df = wpool.tile([128, 2, R], BF16,
                                                tag=f"df{c}", name=f"dfd{sweep}_{n}_{c}")
                                g_prev = g0[c] if (sweep == 0 or n == 1) else st[c][n - 1]
                                if n == N_PLATES:
                                    for ft in range(2):
                                        nc.vector.tensor_scalar(
                                            df[:, ft, :], g_prev[:, ft, :],
                                            ne9_t[:, ft:ft + 1], None, OP.add,
                                        )
                                else:
                                    nc.vector.tensor_tensor(
                                        df, g_prev, st[c][n + 1], OP.subtract)
                                dfs[c] = df
                            mm_block(
                                chs, lambda c: pband[c], wdesc_t, dfs,
                                range(2), range(2),
                                lambda ft, k: (n == N_PLATES and k == 0),
                                lambda ft, k: (n == 1 and k == 1),
                            )
                            for c in chs:
                                e_new = spool.tile([128, 2, R], BF16,
                                                   tag=f"st{n}_{c}",
                                                   name=f"e{sweep}_{n}_{c}")
                                for ft in range(2):
                                    nc.scalar.activation(
                                        e_new[:, ft, :], pband[c][:, ft, :],
                                        AF.Sigmoid,
                                        bias=be_t[:, n - 1, ft:ft + 1],
                                    )
                                st[c][n] = e_new
                            if last:
                                for c in chs:
                                    if n == N_PLATES:
                                        S[c] = spool.tile(
                                            [128, 2, R], BF16, tag=f"S{c}",
                                            name=f"S_{c}")
                                        nc.vector.tensor_copy(S[c], dfs[c])
                                    else:
                                        nc.vector.tensor_tensor(
                                            S[c], S[c], dfs[c], OP.add)
                        return s_desc

                    for n in range(N_PLATES, 0, -1):
                        steps.append(make_desc(n))

                    def make_asc(n, sweep=sweep, last=last):
                        def s_asc():
                            if n == 1:
                                for c in chs:
                                    pband[c] = ppool.tile(
                                        [128, 2, R], F32, tag=f"pp{c}",
                                        name=f"ppa{sweep}_{c}")
                                for ft in range(2):
                                    for c in chs:
                                        nc.tensor.matmul(
                                            pband[c][:, ft, :],
                                            lhsT=iden_t,
                                            rhs=g0[c][:, ft, :],
                                            start=True, stop=False,
                                            skip_group_check=True,
                                        )
                            dfs = {}
                            for c in chs:
                                df = wpool.tile([128, 2, R], BF16,
                                                tag=f"df{c}", name=f"dfa{sweep}_{n}_{c}")
                                g_prev = g0[c] if n == 1 else st[c][n - 1]
                                nc.vector.tensor_tensor(
                                    df, g_prev, st[c][n], OP.subtract)
                                dfs[c] = df
                            mm_block(
                                chs, lambda c: pband[c], wasc_t, dfs,
                                range(2), range(2),
                                lambda ft, k: False,
                                lambda ft, k: (n == N_PLATES and k == 1),
                            )
                            if n <= N_PLATES - 1:
                                for c in chs:
                                    g_sn = spool.tile([128, 2, R], BF16,
                                                      tag=f"st{n}_{c}",
                                                      name=f"gs{sweep}_{n}_{c}")
                                    for ft in range(2):
                                        nc.scalar.activation(
                                            g_sn[:, ft, :], pband[c][:, ft, :],
                                            AF.Identity,
                                            bias=neps_t[:, n - 1, ft:ft + 1],
                                        )
                                    st[c][n] = g_sn
                            elif last:
                                for c in chs:
                                    g8[c] = spool.tile([128, 2, R], BF16,
                                                       tag=f"g8_{c}",
                                                       name=f"g8_{c}")
                                    for ft in range(2):
                                        nc.scalar.activation(
                                            g8[c][:, ft, :], pband[c][:, ft, :],
                                            AF.Identity,
                                            bias=neps_t[:, n - 1, ft:ft + 1],
                                        )
                        return s_asc

                    for n in range(1, N_PLATES + 1):
                        steps.append(make_asc(n))

                def s_head():
                    for c in chs:
                        pband[c] = ppool.tile([128, 2, R], F32, tag=f"pp{c}",
                                              name=f"pph{c}")
                    mm_block(chs, lambda c: pband[c], w1g_t, g8,
                             range(2), range(2),
                             lambda ft, k: (k == 0), lambda ft, k: False)
                    mm_block(chs, lambda c: pband[c], wfold_t, S,
                             range(2), range(2),
                             lambda ft, k: False, lambda ft, k: (k == 1))
                    for c in chs:
                        h = spool.tile([128, 2, R], BF16, tag=f"h{c}",
                                       name=f"h{c}")
                        for ft in range(2):
                            nc.scalar.activation(
                                h[:, ft, :], pband[c][:, ft, :], AF.Relu,
                                bias=hb_t[:, ft:ft + 1],
                            )
                        hs[c] = h
                steps.append(s_head)

                def make_po(b):
                    def s_po():
                        pos = {}
                        for c in chs:
                            po = ppool.tile([128, 2, R], F32, tag=f"pp{c}",
                                            name=f"po{b}_{c}")
                            for k in range(2):
                                nc.tensor.matmul(
                                    po[:BLK, b % 2, :],
                                    lhsT=w2t_t[:, k, b * BLK:(b + 1) * BLK],
                                    rhs=hs[c][:, k, :],
                                    start=(k == 0), stop=(k == 1),
                                )
                            pos[c] = po
                        for c in chs:
                            stg = wpool.tile([128, R], BF16, tag=f"stg{c}",
                                             bufs=4, name=f"stg{b}_{c}")
                            nc.vector.tensor_copy(
                                stg[:BLK, :], pos[c][:BLK, b % 2, :])
                            nc.sync.dma_start(
                                outT_d[b * BLK:(b + 1) * BLK,
                                       c * R:(c + 1) * R],
                                stg[:BLK, :],
                            )
                    return s_po

                for b in range(NB):
                    steps.append(make_po(b))
                return steps

            A = group_steps((0, 1), 0)
            Bs = group_steps((2, 3), 1)
            for i in range(len(A) + STAGGER):
                if i < len(A):
                    A[i]()
                j = i - STAGGER
                if 0 <= j < len(Bs):
                    Bs[j]()

    nc.compile()
    return nc


def _shard_inputs(inp):
    prep = _preprocess_weights(inp)
    xT = np.ascontiguousarray(
        np.asarray(inp["x"], np.float32).astype(BF).T)   # (512, 16384)
    in_maps = []
    for c in range(N_CORES):
        m = {"xT": np.ascontiguousarray(xT[:, c * ROWS:(c + 1) * ROWS])}
        m.update(prep)
        in_maps.append(m)
    return in_maps


def _gather_output(inp, res):
    outT = np.concatenate(
        [res.results[c]["outT"] for c in range(N_CORES)], axis=1)
    out = outT.T.astype(np.float32) + np.asarray(inp["b2"], np.float32)
    return out


_NC_CACHE = {}


def kernel(**inputs):
    inp = {k: np.asarray(v) for k, v in inputs.items()}
    if "nc" not in _NC_CACHE:
        _NC_CACHE["nc"] = build_nc()
    nc = _NC_CACHE["nc"]
    in_maps = _shard_inputs(inp)
    res = bass_utils.run_bass_kernel_spmd(nc, in_maps, core_ids=list(range(N_CORES)))
    return _gather_output(inp, res)
